# revision 4
# baseline (speedup 1.0000x reference)
"""Trainium2 Bass kernel for nn_ConditionalNormalizingFlow.

RealNVP-style conditional normalizing flow: 6 coupling layers, each
  h  = relu([x_keep, cond] @ W1 + b1)        (B,96)->(B,1024)
  h  = relu(h @ W2 + b2)                     (B,1024)->(B,1024)
  log_s = tanh(h @ Ws + bs); t = h @ Wt + bt (B,1024)->(B,32)x2
  x_chg = x_chg * exp(log_s) + t
  log_det += sum(log_s, axis=1)
  full-batch BatchNorm(x) (training stats), log_det += sum(log|bn_w|)

Distribution: pure data parallelism over 8 NeuronCores (4096 rows each).
Activations live transposed ([feature, batch]) in SBUF, split into the
even-indexed / odd-indexed feature halves (the coupling masks alternate).
Matmuls run in float32r (fp32 with 11-bit mantissa, 4x faster than fp32 on
the PE). BatchNorm batch stats are AllReduced across cores ([32,4] fp32 per
layer). log_det's feature-sum is one exact-fp32 matmul pass at the end.
"""

import numpy as np

import concourse.mybir as mybir
import concourse.tile as tile
from concourse import bacc
from concourse.bass import ds, ts
from concourse.bass_utils import run_bass_kernel_spmd

F32 = mybir.dt.float32
F32R = mybir.dt.float32r
AF = mybir.ActivationFunctionType
OP = mybir.AluOpType

B = 32768
D = 64
C = 64
H = 1024
L = 6
S = 32  # SPLIT: half of D
NIN = 96
EPS = 1e-5

NCORES = 8
BL = B // NCORES          # rows per core
CH = 512                  # batch chunk (matmul moving dim)
NCH = BL // CH            # chunks per core
MT = H // 128             # 8 m-tiles of the hidden dim
KT = H // 128             # 8 k-tiles of the hidden dim


def _build(n_cores=NCORES, collectives=True):
    import os
    n_layers = int(os.environ.get("KERNEL_LAYERS", L))
    nc = bacc.Bacc("TRN2", target_bir_lowering=False, debug=False,
                   num_devices=n_cores)

    # ---- DRAM inputs (per core). Matmul operands declared float32r. ----
    xe_d = nc.dram_tensor("xe", [S, BL], F32R, kind="ExternalInput").ap()
    xo_d = nc.dram_tensor("xo", [S, BL], F32R, kind="ExternalInput").ap()
    ct_d = nc.dram_tensor("ct", [C, BL], F32R, kind="ExternalInput").ap()
    w1_d = nc.dram_tensor("w1", [L, NIN, H], F32R, kind="ExternalInput").ap()
    w2_d = nc.dram_tensor("w2", [L, H, H], F32R, kind="ExternalInput").ap()
    wst_d = nc.dram_tensor("wst", [L, H, 2 * S], F32R, kind="ExternalInput").ap()
    b1_d = nc.dram_tensor("b1c", [128, L * MT], F32, kind="ExternalInput").ap()
    b2_d = nc.dram_tensor("b2c", [128, L * MT], F32, kind="ExternalInput").ap()
    bst_d = nc.dram_tensor("bstc", [2 * S, L], F32, kind="ExternalInput").ap()
    bnw_d = nc.dram_tensor("bnwc", [S, 2 * L], F32, kind="ExternalInput").ap()
    bnb_d = nc.dram_tensor("bnbc", [S, 2 * L], F32, kind="ExternalInput").ap()
    ldc_d = nc.dram_tensor("ldc", [1, 1], F32, kind="ExternalInput").ap()

    xe_o = nc.dram_tensor("xe_out", [S, BL], F32R, kind="ExternalOutput").ap()
    xo_o = nc.dram_tensor("xo_out", [S, BL], F32R, kind="ExternalOutput").ap()
    ld_o = nc.dram_tensor("ld_out", [1, BL], F32, kind="ExternalOutput").ap()

    with tile.TileContext(nc) as tc:
        with (
            tc.tile_pool(name="const", bufs=1) as cp,
            tc.tile_pool(name="w1p", bufs=2) as w1p,
            tc.tile_pool(name="w2p", bufs=2) as w2p,
            tc.tile_pool(name="wstp", bufs=2) as wstp,
            tc.tile_pool(name="h1p", bufs=1) as h1p,
            tc.tile_pool(name="h2p", bufs=1) as h2p,
            tc.tile_pool(name="scp", bufs=2) as scp,
            tc.tile_pool(name="smp", bufs=2) as smp,
            tc.tile_pool(name="ph1", bufs=2, space="PSUM") as ph1p,
            tc.tile_pool(name="ph2", bufs=2, space="PSUM") as ph2p,
            tc.tile_pool(name="pst", bufs=2, space="PSUM") as pstp,
            tc.tile_pool(name="pld", bufs=2, space="PSUM") as pldp,
            tc.tile_pool(name="dram", bufs=1, space="DRAM") as dramp,
        ):
            # ---- persistent tiles ----
            TE = cp.tile([NIN, BL], F32R, tag="TE")   # rows 0-31 evens, 32-95 cond
            TO = cp.tile([NIN, BL], F32R, tag="TO")   # rows 0-31 odds,  32-95 cond
            nc.sync.dma_start(TE[0:S, :], xe_d)
            nc.sync.dma_start(TO[0:S, :], xo_d)
            nc.sync.dma_start(TE[S:NIN, :], ct_d)
            nc.sync.dma_start(TO[S:NIN, :], ct_d)

            b1s = cp.tile([128, L * MT], F32, tag="b1s")
            b2s = cp.tile([128, L * MT], F32, tag="b2s")
            bsts = cp.tile([2 * S, L], F32, tag="bsts")
            bnw = cp.tile([S, 2 * L], F32, tag="bnw")  # col 2i: evens, 2i+1: odds
            bnb = cp.tile([S, 2 * L], F32, tag="bnb")
            ldc = cp.tile([1, 1], F32, tag="ldc")
            nc.sync.dma_start(b1s[:], b1_d)
            nc.sync.dma_start(b2s[:], b2_d)
            nc.sync.dma_start(bsts[:], bst_d)
            nc.sync.dma_start(bnw[:], bnw_d)
            nc.sync.dma_start(bnb[:], bnb_d)
            nc.sync.dma_start(ldc[:], ldc_d)

            ls_acc = cp.tile([S, BL], F32, tag="ls_acc")
            nc.vector.memset(ls_acc[:], 0.0)
            ones = cp.tile([S, 1], F32, tag="ones")
            nc.vector.memset(ones[:], 1.0)

            # per-chunk stat partials: [32, NCH] per half per moment
            pS1 = {}
            pS2 = {}
            for h in "eo":
                pS1[h] = cp.tile([S, NCH], F32, tag=f"pS1{h}", name=f"pS1{h}")
                pS2[h] = cp.tile([S, NCH], F32, tag=f"pS2{h}", name=f"pS2{h}")

            inv_b = 1.0 / (B if collectives else BL)

            for i in range(n_layers):
                keep = TE if i % 2 == 0 else TO
                chg = TO if i % 2 == 0 else TE

                # ---- layer weights (double-buffered pools) ----
                w1t = w1p.tile([NIN, MT, 128], F32R, tag="w1t")
                nc.sync.dma_start(
                    w1t[:], w1_d[i].rearrange("k (mt m) -> k mt m", m=128))
                w2t = w2p.tile([128, KT, H], F32R, tag="w2t")
                for k in range(KT):
                    nc.sync.dma_start(w2t[:, k, :], w2_d[i, ds(k * 128, 128), :])
                wstt = wstp.tile([128, KT, 2 * S], F32R, tag="wstt")
                nc.sync.dma_start(
                    wstt[:], wst_d[i].rearrange("(kt p) m -> p kt m", p=128))

                for n in range(NCH):
                    cs = ds(n * CH, CH)
                    # ---- stage A: h1 = relu(W1^T [keep;cond] + b1) ----
                    h1 = h1p.tile([128, MT, CH], F32R, tag="h1")
                    for m in range(MT):
                        ph = ph1p.tile([128, CH], F32, tag="ph1")
                        nc.tensor.matmul(ph[:], w1t[:, m, :], keep[:, cs],
                                         start=True, stop=True)
                        bcol = b1s[:, i * MT + m: i * MT + m + 1]
                        if m % 2 == 0:
                            nc.scalar.activation(h1[:, m, :], ph[:], AF.Relu,
                                                 bias=bcol)
                        else:
                            nc.vector.tensor_scalar(h1[:, m, :], ph[:], bcol,
                                                    0.0, OP.add, OP.max)
                    # ---- stage B: h2 = relu(W2^T h1 + b2) ----
                    h2 = h2p.tile([128, MT, CH], F32R, tag="h2")
                    for m in range(MT):
                        ph = ph2p.tile([128, CH], F32, tag="ph2")
                        for k in range(KT):
                            nc.tensor.matmul(ph[:], w2t[:, k, ds(m * 128, 128)],
                                             h1[:, k, :],
                                             start=(k == 0), stop=(k == KT - 1))
                        bcol = b2s[:, i * MT + m: i * MT + m + 1]
                        if m % 2 == 0:
                            nc.scalar.activation(h2[:, m, :], ph[:], AF.Relu,
                                                 bias=bcol)
                        else:
                            nc.vector.tensor_scalar(h2[:, m, :], ph[:], bcol,
                                                    0.0, OP.add, OP.max)
                    # ---- stage C: [log_s_pre; t_pre] = WST^T h2 ----
                    pstt = pstp.tile([2 * S, CH], F32, tag="pst")
                    for k in range(KT):
                        nc.tensor.matmul(pstt[:], wstt[:, k, :], h2[:, k, :],
                                         start=(k == 0), stop=(k == KT - 1))
                    ls = scp.tile([S, CH], F32, tag="ls")
                    nc.scalar.activation(ls[:], pstt[0:S, :], AF.Tanh,
                                         bias=bsts[0:S, i:i + 1])
                    ex = scp.tile([S, CH], F32, tag="ex")
                    nc.scalar.activation(ex[:], ls[:], AF.Exp)
                    tt = scp.tile([S, CH], F32, tag="tt")
                    nc.vector.tensor_scalar_add(tt[:], pstt[S:2 * S, :],
                                                bsts[S:2 * S, i:i + 1])
                    # ---- x_chg = x_chg * exp(log_s) + t (in place) ----
                    tmp = scp.tile([S, CH], F32, tag="tmp")
                    nc.vector.tensor_tensor(tmp[:], chg[0:S, cs], ex[:], OP.mult)
                    nc.vector.tensor_tensor(chg[0:S, cs], tmp[:], tt[:], OP.add)
                    # ---- log_det accumulation ----
                    nc.vector.tensor_tensor(ls_acc[:, cs], ls_acc[:, cs], ls[:],
                                            OP.add)
                    # ---- per-chunk BN stat partials for both halves ----
                    for hname, T in (("e", TE), ("o", TO)):
                        nc.vector.reduce_sum(pS1[hname][:, n:n + 1], T[0:S, cs],
                                             axis=mybir.AxisListType.X)
                        sq = scp.tile([S, CH], F32, tag=f"sq{hname}")
                        nc.scalar.activation(sq[:], T[0:S, cs], AF.Square,
                                             accum_out=pS2[hname][:, n:n + 1])

                # ---- reduce partials -> [32,4]; AllReduce; BN params ----
                stats = smp.tile([S, 4], F32, tag="stats")
                for j, (mom, h) in enumerate(
                        ((pS1, "e"), (pS1, "o"), (pS2, "e"), (pS2, "o"))):
                    nc.vector.reduce_sum(stats[:, j:j + 1], mom[h][:],
                                         axis=mybir.AxisListType.X)
                if collectives:
                    ar_in = dramp.tile([S, 4], F32, tag=f"ari{i}")
                    ar_out = dramp.tile([S, 4], F32, tag=f"aro{i}")
                    nc.gpsimd.dma_start(ar_in[:], stats[:])
                    nc.gpsimd.collective_compute(
                        "AllReduce", OP.add,
                        replica_groups=[list(range(n_cores))],
                        ins=[ar_in.opt()], outs=[ar_out.opt()])
                    sg = smp.tile([S, 4], F32, tag="sg")
                    nc.gpsimd.dma_start(sg[:], ar_out[:])
                else:
                    sg = stats

                ab = smp.tile([S, 4], F32, tag="ab")  # cols: a_e, c_e, a_o, c_o
                for j, h in enumerate("eo"):
                    mean = smp.tile([S, 1], F32, tag=f"mean{h}")
                    nc.vector.tensor_scalar_mul(mean[:], sg[:, j:j + 1], inv_b)
                    var = smp.tile([S, 1], F32, tag=f"var{h}")
                    nc.vector.tensor_scalar_mul(var[:], sg[:, 2 + j:3 + j], inv_b)
                    msq = smp.tile([S, 1], F32, tag=f"msq{h}")
                    nc.vector.tensor_tensor(msq[:], mean[:], mean[:], OP.mult)
                    nc.vector.tensor_tensor(var[:], var[:], msq[:], OP.subtract)
                    nc.vector.tensor_scalar_add(var[:], var[:], EPS)
                    rv = smp.tile([S, 1], F32, tag=f"rv{h}")
                    nc.vector.reciprocal(rv[:], var[:])
                    rstd = smp.tile([S, 1], F32, tag=f"rstd{h}")
                    nc.scalar.activation(rstd[:], rv[:], AF.Sqrt)
                    nc.vector.tensor_tensor(ab[:, 2 * j:2 * j + 1], rstd[:],
                                            bnw[:, 2 * i + j:2 * i + j + 1],
                                            OP.mult)
                    ma = smp.tile([S, 1], F32, tag=f"ma{h}")
                    nc.vector.tensor_tensor(ma[:], mean[:],
                                            ab[:, 2 * j:2 * j + 1], OP.mult)
                    nc.vector.tensor_tensor(ab[:, 2 * j + 1:2 * j + 2],
                                            bnb[:, 2 * i + j:2 * i + j + 1],
                                            ma[:], OP.subtract)

                # ---- apply BN to both halves, per chunk ----
                for n in range(NCH):
                    cs = ds(n * CH, CH)
                    nc.vector.tensor_scalar(TE[0:S, cs], TE[0:S, cs],
                                            ab[:, 0:1], ab[:, 1:2],
                                            OP.mult, OP.add)
                    nc.vector.tensor_scalar(TO[0:S, cs], TO[0:S, cs],
                                            ab[:, 2:3], ab[:, 3:4],
                                            OP.mult, OP.add)

            # ---- log_det = ones^T ls_acc + ldc (exact fp32 matmul) ----
            ldsb = cp.tile([1, BL], F32, tag="ldsb")
            for n in range(NCH):
                cs = ds(n * CH, CH)
                pl = pldp.tile([1, CH], F32, tag="pld")
                nc.tensor.matmul(pl[:], ones[:], ls_acc[:, cs],
                                 start=True, stop=True)
                nc.vector.tensor_scalar_add(ldsb[:, cs], pl[:], ldc[:])

            nc.sync.dma_start(xe_o, TE[0:S, :])
            nc.sync.dma_start(xo_o, TO[0:S, :])
            nc.sync.dma_start(ld_o, ldsb[:])

    nc.compile()
    return nc


_cache = {}


def _get_nc():
    if "nc" not in _cache:
        _cache["nc"] = _build(NCORES, True)
    return _cache["nc"]


def _prep_weights(W1, b1, W2, b2, Ws, bs, Wt, bt, bn_w, bn_b):
    W1 = np.ascontiguousarray(W1, np.float32)
    W2 = np.ascontiguousarray(W2, np.float32)
    wst = np.ascontiguousarray(np.concatenate([Ws, Wt], axis=2), np.float32)
    b1c = np.ascontiguousarray(
        np.asarray(b1, np.float32).reshape(L, MT, 128).transpose(2, 0, 1)
        .reshape(128, L * MT))
    b2c = np.ascontiguousarray(
        np.asarray(b2, np.float32).reshape(L, MT, 128).transpose(2, 0, 1)
        .reshape(128, L * MT))
    bstc = np.ascontiguousarray(
        np.concatenate([bs, bt], axis=1).T.astype(np.float32))  # [64, L]
    bnw = np.asarray(bn_w, np.float32)
    bnb = np.asarray(bn_b, np.float32)
    # [S, 2L]: col 2i evens of layer i, col 2i+1 odds
    bnwc = np.empty((S, 2 * L), np.float32)
    bnbc = np.empty((S, 2 * L), np.float32)
    for i in range(L):
        bnwc[:, 2 * i] = bnw[i, 0::2]
        bnwc[:, 2 * i + 1] = bnw[i, 1::2]
        bnbc[:, 2 * i] = bnb[i, 0::2]
        bnbc[:, 2 * i + 1] = bnb[i, 1::2]
    ldc = np.array([[np.log(np.abs(bnw)).sum()]], np.float32)
    return dict(w1=W1, w2=W2, wst=wst, b1c=b1c, b2c=b2c, bstc=bstc,
                bnwc=bnwc, bnbc=bnbc, ldc=ldc)


def kernel(z, cond, W1, b1, W2, b2, Ws, bs, Wt, bt, bn_w, bn_b):
    z = np.asarray(z, np.float32)
    cond = np.asarray(cond, np.float32)
    wmap = _prep_weights(W1, b1, W2, b2, Ws, bs, Wt, bt, bn_w, bn_b)

    in_maps = []
    for c in range(NCORES):
        rows = slice(c * BL, (c + 1) * BL)
        zc = z[rows]
        in_maps.append(dict(
            xe=np.ascontiguousarray(zc[:, 0::2].T),
            xo=np.ascontiguousarray(zc[:, 1::2].T),
            ct=np.ascontiguousarray(cond[rows].T),
            **wmap,
        ))

    nc = _get_nc()
    res = run_bass_kernel_spmd(nc, in_maps, core_ids=list(range(NCORES)))

    x = np.empty((B, D), np.float32)
    log_det = np.empty((B,), np.float32)
    for c in range(NCORES):
        rows = slice(c * BL, (c + 1) * BL)
        r = res.results[c]
        x[rows, 0::2] = r["xe_out"].T
        x[rows, 1::2] = r["xo_out"].T
        log_det[rows] = r["ld_out"][0]
    return x, log_det
